# revision 16
# baseline (speedup 1.0000x reference)
"""Cross-attention Trainium2 Bass kernel.

Reference computation (per batch b):
    q = relu(scale_q * (Wq @ qf) + bias_q)          [C, Nq]
    k = relu(scale_k * (Wk @ kf) + bias_k)          [C, Nk]
    v = relu(scale_v * (Wv @ kf) + bias_v)          [C, Nk]
    sim  = q.T @ k / sqrt(C)                        [Nq, Nk]
    attn = softmax(sim, axis=-1)
    ctx  = v @ attn.T                               [C, Nq]

Sharding: 8 cores = 4 batches x 2 query halves (Nq 4096 -> 2048 per core).
Each core gets the full K/V for its batch (recomputed, cheap) and half the
query positions; output halves are concatenated (and transposed) on the host.

Device-side design (per core) -- everything quantizes to fp8e4m3, which the
softmax renormalization + averaging over 4096 keys washes out (measured
~4e-3 relative Frobenius error against the 2e-2 budget):
  - BN scale folded into the weights on the host; weights fed pre-transposed,
    fp8, and pre-packed into the SBUF per-partition layout so every weight
    DMA is one contiguous run per partition. Inputs stream as fp8 (half the
    bytes of the old fp32r pipeline).
  - All projection matmuls are DoubleRow fp8: the [128ci, 2co, *] stationary
    contracts all 256 input channels in one pass at 2 MACs/cell/cycle.
    q/k biases ride free on the ACT/DVE relu ops (per-partition); the v bias
    varies along the free dim, so it is added from a host-broadcast tile on
    GpSimd, with the relu alternating ACT/DVE -- three engines share the
    elementwise work so none of them rate-limits PE.
  - sim is computed transposed (k on partitions, q on free dim) as one
    DoubleRow matmul per key tile. Softmax uses a constant shift instead of
    a row max: exp(sim/sqrt(C) - 4) (sim/sqrt(C) is bounded by ~|q||k|/16
    << 88, so no overflow; q,k >= 0 post-relu so sim >= 0).
  - exp alternates between two engines: even key-pairs run exact exp on ACT
    (scale+bias ride free on the activation); odd pairs run on DVE as a
    single multiply-add producing the fp8 BIT PATTERN directly
    (Schraudolph in the fp8 domain: bits = round(sim*scale*8*log2e + (56 +
    8*log2e*SHIFT)), written through a uint8 bitcast view). The piecewise-
    log-linear error is common-mode across each softmax row and cancels in
    the normalization (measured: it is no worse than exact-exp-then-
    quantize). Together the two engines produce exp faster than PE consumes
    it, so attention runs at the PE DoubleRow roofline.
  - ctx runs TRANSPOSED as DoubleRow fp8: stationary = exp pair [128k, 2kt,
    128q], moving = v^T pair [128k, 2kt, 257] whose column 256 is an
    all-ones channel, so each PSUM accumulator [128q, 257] collects the 256
    context channels AND the softmax denominator in the same pass -- no
    separate row-sum matmuls, and each matmul covers TWO key tiles.
  - Normalization is per-partition (q on partitions): reciprocal_approx_fast
    on the denominator column [128,1] + one tensor_scalar multiply, then a
    per-subtile output DMA.
  - Flat software pipeline: the sim/exp stream runs at k-pair granularity;
    each ctx quadruplet (4 DoubleRow matmuls for one accumulator) issues 6
    pairs behind the exp that feeds it, so the fill/drain bubbles are ~6
    pairs instead of a full chunk.
  - Per-core output is [Nq, C] (q-major); the host transposes.
"""

import sys

for _p in ("/opt/trn_rl_repo", "/root/.axon_site/_ro/trn_rl_repo"):
    if _p not in sys.path:
        sys.path.insert(0, _p)

import numpy as np

import concourse.bacc as bacc
import concourse.mybir as mybir
import concourse.tile as tile
from concourse.bass_utils import run_bass_kernel_spmd

F32 = mybir.dt.float32
FP8 = mybir.dt.float8e4
U8 = mybir.dt.uint8
AF = mybir.ActivationFunctionType
DR = mybir.MatmulPerfMode.DoubleRow

B, C, H, W = 4, 256, 64, 64
NK = H * W          # 4096 key positions per batch
NQ = NK // 2        # 2048 query positions per core
P = 128
CO = 2              # contraction subtiles (C = 2*128)
QC = 512            # query chunk (sim moving free dim)
NQC = NQ // QC      # 4 query chunks per core
KT = NK // P        # 32 key tiles
NP = KT // 2        # 16 key-pair tiles per chunk
QT = QC // P        # 4 q-subtiles per chunk (ctx accumulators)
VF = 272            # vT/wvT free-dim pitch (>=258, 16B-aligned for DoubleRow)
LAG = 6             # ctx quadruplets trail the sim/exp stream by 6 pairs
EXP_SHIFT = -4.0    # exp(sim/sqrt(C) + EXP_SHIFT); sim/sqrt(C) observed in [0.5, 7.5]
SCALE = 1.0 / np.sqrt(C)
A8 = 8.0 * np.log2(np.e)                  # fp8e4m3 bits per e-fold
DVE_EXP_MUL = float(SCALE * A8)           # sim -> fp8 bits multiplier
DVE_EXP_ADD = float(56.0 + A8 * EXP_SHIFT)  # 8*bias(7) + shift term


def _build_program():
    nc = bacc.Bacc("TRN2", target_bir_lowering=False, debug=False)

    qf = nc.dram_tensor("qf", [C, NQ], FP8, kind="ExternalInput").ap()
    kf = nc.dram_tensor("kf", [C, NK], FP8, kind="ExternalInput").ap()
    wqT = nc.dram_tensor("wqT", [P, CO * C], FP8, kind="ExternalInput").ap()
    wkT = nc.dram_tensor("wkT", [P, CO * C], FP8, kind="ExternalInput").ap()
    wvT = nc.dram_tensor("wvT", [P, CO * VF], FP8, kind="ExternalInput").ap()
    bq = nc.dram_tensor("bq", [P, CO], F32, kind="ExternalInput").ap()
    bk = nc.dram_tensor("bk", [P, CO], F32, kind="ExternalInput").ap()
    bv8 = nc.dram_tensor("bv8", [1, VF], FP8, kind="ExternalInput").ap()
    out = nc.dram_tensor("out", [NQ, C], F32, kind="ExternalOutput").ap()
    out_t = out.rearrange("(g p) c -> p g c", p=P)   # [128, 16, 256]

    with tile.TileContext(nc) as tc:
        with (
            nc.allow_low_precision(reason="fp8 matmul pipeline"),
            tc.tile_pool(name="consts", bufs=1) as consts,
            tc.tile_pool(name="persist", bufs=1) as persist,
        ):
            # ---- constants (issue order matters: the first projection only
            # needs wqT + bq + qf, so those go first) ----
            wqT_sb = consts.tile([P, CO, C], FP8, name="wqT_sb")
            nc.gpsimd.dma_start(wqT_sb[:], wqT.rearrange("p (co o) -> p co o", co=CO))
            bq_sb = consts.tile([P, CO], F32, name="bq_sb")
            wkT_sb = consts.tile([P, CO, C], FP8, name="wkT_sb")
            wvT_sb = consts.tile([P, CO, VF], FP8, name="wvT_sb")
            bk_sb = consts.tile([P, CO], F32, name="bk_sb")
            bv8_sb = consts.tile([1, VF], FP8, name="bv8_sb")
            ones1 = consts.tile([1, P], FP8, name="ones1")
            nc.vector.memset(ones1[:], 1.0)
            b0_sb = consts.tile([P, 1], F32, name="b0_sb")
            nc.vector.memset(b0_sb[:], EXP_SHIFT)
            # dummy activation: pulls the ~1.3us LoadActFuncSet into the
            # initial DMA-wait window instead of blocking the first relu
            warm_sb = consts.tile([P, 1], F32, name="warm_sb")
            nc.scalar.activation(warm_sb[:], b0_sb[:], AF.Relu)

            # ---- persistent activations (fp8) ----
            q_sb = persist.tile([P, CO, NQ], FP8, name="q_sb")
            k_sb = persist.tile([P, CO, NK], FP8, name="k_sb")
            vT_sb = persist.tile([P, KT, VF], FP8, name="vT_sb")

            # ---- projections (staging pool scoped so its SBUF is reused) ----
            with (
                tc.tile_pool(name="staging", bufs=1) as staging,
                tc.tile_pool(name="proj_ps", bufs=1, space="PSUM") as proj_ps,
            ):
                qf_sb = staging.tile([P, CO, NQ], FP8, name="qf_sb")
                qf_t = qf.rearrange("(co ci) n -> ci co n", ci=P)
                kf_sb = staging.tile([P, CO, NK], FP8, name="kf_sb")
                kf_t = kf.rearrange("(co ci) n -> ci co n", ci=P)
                nc.gpsimd.dma_start(bq_sb[:], bq[:])
                nc.gpsimd.dma_start(wkT_sb[:], wkT.rearrange("p (co o) -> p co o", co=CO))
                nc.gpsimd.dma_start(bk_sb[:], bk[:])
                nc.gpsimd.dma_start(wvT_sb[:], wvT.rearrange("p (co o) -> p co o", co=CO))
                nc.gpsimd.dma_start(bv8_sb[:], bv8[:])
                nc.sync.dma_start(qf_sb[:], qf_t)
                nc.sync.dma_start(kf_sb[:, :, :NK // 2], kf_t[:, :, :NK // 2])
                nc.sync.dma_start(kf_sb[:, :, NK // 2:], kf_t[:, :, NK // 2:])

                def proj_iter(j, w_sb, bias_sb, dst, src_sb):
                    # one [*, QC] chunk of a q/k projection: a single
                    # DoubleRow matmul per 128-channel output block, then
                    # relu+bias+fp8 cast on ACT (oo=0) / DVE (oo=1)
                    for oo in range(CO):
                        ps = proj_ps.tile([P, QC], F32, tag="pj", bufs=2,
                                          name=f"ps_{j}_{oo}")
                        nc.tensor.matmul(
                            ps[:],
                            w_sb[:, :, oo * P:(oo + 1) * P],
                            src_sb[:, :, j * QC:(j + 1) * QC],
                            start=True, stop=True, perf_mode=DR,
                        )
                        if oo == 0:
                            nc.scalar.activation(
                                dst[:, oo, j * QC:(j + 1) * QC], ps[:], AF.Relu,
                                bias=bias_sb[:, oo:oo + 1],
                            )
                        else:
                            nc.vector.tensor_scalar(
                                dst[:, oo, j * QC:(j + 1) * QC], ps[:],
                                bias_sb[:, oo:oo + 1], 0.0,
                                mybir.AluOpType.add, mybir.AluOpType.max,
                            )

                def vt_pair(kp):
                    # vT = relu(kf.T @ Wv'.T + bias_v): [n, o], n on
                    # partitions; column C is the ones channel (0-weight col
                    # + bias 1.0). bias_v varies along the free dim, so a
                    # K=1 matmul (ones column x bias row) initializes the
                    # PSUM accumulator and the DoubleRow matmul accumulates
                    # on top; relu + fp8 cast straight from PSUM alternates
                    # ACT/DVE so neither engine rate-limits PE.
                    psv = proj_ps.tile([P, 2, QC], F32, tag="pv", bufs=3,
                                       name=f"psv_{kp}")
                    for half in range(2):
                        kt = 2 * kp + half
                        nc.tensor.matmul(
                            psv[:, half, :C + 2],
                            ones1[:], bv8_sb[:, :C + 2],
                            start=True, stop=False, skip_group_check=True,
                        )
                        nc.tensor.matmul(
                            psv[:, half, :C + 2],
                            kf_sb[:, :, kt * P:(kt + 1) * P],
                            wvT_sb[:, :, :C + 2],
                            start=False, stop=True, perf_mode=DR,
                            skip_group_check=True,
                        )
                    dst = vT_sb[:, 2 * kp:2 * kp + 2, :C + 2]
                    if kp % 2 == 0:
                        nc.scalar.activation(dst, psv[:, :, :C + 2], AF.Relu)
                    else:
                        nc.vector.tensor_relu(dst, psv[:, :, :C + 2])

                for j in range(NQ // QC):
                    proj_iter(j, wqT_sb, bq_sb, q_sb, qf_sb)
                for j in range(NK // QC):
                    proj_iter(j, wkT_sb, bk_sb, k_sb, kf_sb)
                    vt_pair(2 * j)
                    vt_pair(2 * j + 1)

            # ---- attention ----
            with (
                tc.tile_pool(name="expp", bufs=1) as expp,
                tc.tile_pool(name="outp", bufs=1) as outp,
                tc.tile_pool(name="attn_ps", bufs=1, space="PSUM") as attn_ps,
            ):
                e_pairs = {}    # qc -> list of 16 fp8 pair tiles
                out_tiles = {}  # qc -> [P, QT, C] staging tile
                ctx_tiles = {}  # (qc, qt) -> PSUM accumulator

                def emit_sim_pair(qc, kp):
                    # 2 DoubleRow matmuls (each contracts all 256 channels),
                    # then exp: exact on ACT for even pairs, fp8-domain
                    # Schraudolph on DVE for odd pairs
                    qs = slice(qc * QC, (qc + 1) * QC)
                    ps = attn_ps.tile([P, 2, QC], F32, tag="sim", bufs=2,
                                      name=f"pss_{qc}_{kp}")
                    for half in range(2):
                        kt = 2 * kp + half
                        nc.tensor.matmul(
                            ps[:, half, :],
                            k_sb[:, :, kt * P:(kt + 1) * P],
                            q_sb[:, :, qs],
                            start=True, stop=True, perf_mode=DR,
                        )
                    et = expp.tile([P, 2, QC], FP8, tag="expT", bufs=32,
                                   name=f"expT_{qc}_{kp}")
                    if kp % 2 == 0:
                        nc.scalar.activation(et[:], ps[:], AF.Exp,
                                             bias=b0_sb[:], scale=float(SCALE))
                    else:
                        nc.vector.tensor_scalar(
                            et[:].bitcast(U8), ps[:],
                            DVE_EXP_MUL, DVE_EXP_ADD,
                            mybir.AluOpType.mult, mybir.AluOpType.add,
                        )
                    e_pairs.setdefault(qc, []).append(et)

                def emit_ctx_job(qc, j):
                    # job j of chunk qc: 4 DoubleRow matmuls (key pairs
                    # quarter*4..quarter*4+4) into accumulator qt; col C of
                    # the moving vT pair is the ones channel -> denominator
                    quarter, qt = divmod(j, QT)
                    if quarter == 0:
                        ctx_tiles[(qc, qt)] = attn_ps.tile(
                            [P, C + 1], F32, tag="ctx", bufs=4,
                            name=f"psc_{qc}_{qt}")
                    ctx_ps = ctx_tiles[(qc, qt)]
                    qoff = qt * P
                    for kp in range(quarter * 4, quarter * 4 + 4):
                        nc.tensor.matmul(
                            ctx_ps[:],
                            e_pairs[qc][kp][:, :, qoff:qoff + P],
                            vT_sb[:, 2 * kp:2 * kp + 2, :C + 1],
                            start=(kp == 0), stop=(kp == NP - 1),
                            perf_mode=DR, skip_group_check=True,
                        )
                    if quarter == QT - 1:
                        emit_norm(qc, qt)

                def emit_norm(qc, qt):
                    ctx_ps = ctx_tiles.pop((qc, qt))
                    recip = outp.tile([P, 1], F32, tag="recip", bufs=4,
                                      name=f"recip_{qc}_{qt}")
                    nc.vector.reciprocal_approx_fast(recip[:],
                                                     ctx_ps[:, C:C + 1])
                    ob = out_tiles[qc]
                    nc.vector.tensor_scalar_mul(ob[:, qt, :], ctx_ps[:, :C],
                                                recip[:])
                    nc.sync.dma_start(out_t[:, qc * QT + qt, :], ob[:, qt, :])

                # flat pipeline over global pair slots: slot t runs sim pair
                # (t//NP, t%NP) and ctx job t-LAG (quarter-major, 4 matmuls);
                # the last LAG slots drain the remaining ctx jobs
                TOT = NQC * NP
                for t in range(TOT + LAG):
                    if t < TOT:
                        qc, kp = divmod(t, NP)
                        if kp == 0:
                            out_tiles[qc] = outp.tile(
                                [P, QT, C], F32, tag="ob", bufs=2,
                                name=f"ob_{qc}")
                        emit_sim_pair(qc, kp)
                    tj = t - LAG
                    if tj >= 0:
                        emit_ctx_job(*divmod(tj, NP))

    nc.compile()
    return nc


_PROGRAM = None


def _get_program():
    global _PROGRAM
    if _PROGRAM is None:
        _PROGRAM = _build_program()
    return _PROGRAM


def _prepare_in_maps(
    query_feats, key_feats, Wq, Wk, Wv,
    scale_q, bias_q, scale_k, bias_k, scale_v, bias_v,
):
    f32 = np.float32
    fp8 = mybir.dt.np(FP8)
    qf_all = np.asarray(query_feats, f32).reshape(B, C, NK)
    kf_all = np.asarray(key_feats, f32).reshape(B, C, NK)

    def pack_w(wT, pitch=None):
        # [C, M] weight -> fp8 [P, CO*pitch]: row ci holds the co=0 then
        # co=1 slab, matching the SBUF tile layout [ci][co][o]
        m = wT.shape[1]
        pitch = pitch or m
        p3 = np.zeros((P, CO, pitch), f32)
        p3[:, :, :m] = wT.reshape(CO, P, m).transpose(1, 0, 2)
        return np.ascontiguousarray(p3.reshape(P, CO * pitch)).astype(fp8)

    wqT = pack_w(np.ascontiguousarray(
        (np.asarray(scale_q, f32)[:, None] * np.asarray(Wq, f32)).T))
    wkT = pack_w(np.ascontiguousarray(
        (np.asarray(scale_k, f32)[:, None] * np.asarray(Wk, f32)).T))
    wvT_2d = np.zeros((C, C + 2), f32)
    wvT_2d[:, :C] = np.ascontiguousarray(
        (np.asarray(scale_v, f32)[:, None] * np.asarray(Wv, f32)).T)
    wvT = pack_w(wvT_2d, pitch=VF)
    bq2 = np.ascontiguousarray(np.asarray(bias_q, f32).reshape(CO, P).T)
    bk2 = np.ascontiguousarray(np.asarray(bias_k, f32).reshape(CO, P).T)
    bv8 = np.zeros((1, VF), f32)
    bv8[0, :C] = np.asarray(bias_v, f32)
    bv8[0, C] = 1.0
    bv8 = bv8.astype(fp8)

    shared = dict(wqT=wqT, wkT=wkT, wvT=wvT, bq=bq2, bk=bk2, bv8=bv8)
    kf8 = [np.ascontiguousarray(kf_all[b]).astype(fp8) for b in range(B)]
    in_maps = []
    for core in range(8):
        b, h = divmod(core, 2)
        in_maps.append(dict(
            qf=np.ascontiguousarray(
                qf_all[b][:, h * NQ:(h + 1) * NQ]).astype(fp8),
            kf=kf8[b],
            **shared,
        ))
    return in_maps


def run(inputs: dict, trace: bool = False):
    """Compile (cached) + run on 8 cores. Returns (output, BassKernelResults)."""
    nc = _get_program()
    in_maps = _prepare_in_maps(**inputs)
    res = run_bass_kernel_spmd(nc, in_maps, core_ids=list(range(8)), trace=trace)
    full = np.empty((B, C, NK), np.float32)
    for core in range(8):
        b, h = divmod(core, 2)
        full[b][:, h * NQ:(h + 1) * NQ] = res.results[core]["out"].T
    return full.reshape(B, C, H, W), res


def kernel(**inputs) -> np.ndarray:
    return run(inputs)[0]
